# revision 17
# baseline (speedup 1.0000x reference)
"""Trainium2 Bass kernel for nn_NoisyTopkRouter.

Computes, for x = mh_output [8, 4096, 1024], noise [8, 4096, 64]:
    logits   = x @ W_route.T + b_route        # [.., 64]
    nlogits  = x @ W_noise.T + b_noise
    noisy    = logits + noise * softplus(nlogits)
    top2 vals/idx, softmax over top2, scattered back to 64 experts.
Returns (router_output [8,4096,64] f32, indices [8,4096,2] int32).

Sharding: batch/token-parallel — core i handles mh_output[i] (4096 tokens).

Device layout (per core):
  - x is host-pre-transposed to xT [1024, 4096] so d_model lands on SBUF
    partitions (PE contracts over partitions).
  - Wcat = [W_route; W_noise] (128 experts-rows) is the stationary matmul
    operand; moving operand is xT in N=512 token tiles. PSUM gets
    [128(=2*64 experts), 512 tokens] per tile: rows 0:64 route logits,
    rows 64:128 noise logits.
  - softplus via Exp + Ln(1+e) (single ACT table set), noisy computed
    expert-major, PE-transposed back to token-major [128 tokens, 64 experts]
    for DVE top-8/max_index, 2-way softmax, and iota-compare scatter.
"""

import os
import numpy as np

N_CORES = 8
D = 1024
E = 64
T = 4096  # tokens per core
TT = 512  # tokens per tile
NT = T // TT  # tiles per core
G = T // 128  # 128-token groups per core (32)

_cache: dict = {}


def _build(mm_dtype_name: str, *, xp_bufs=2, ps_bufs=2, work_bufs=2, do_compile=True, ablate="full", ntok_eng="vector", scat_eng="vector", split_x=1, lag=0, reps=0):
    from contextlib import ExitStack

    import concourse.mybir as mybir
    import concourse.tile as tile
    from concourse import bacc

    f32 = mybir.dt.float32
    u32 = mybir.dt.uint32
    mm_dt = getattr(mybir.dt, mm_dtype_name, mybir.dt.float32)
    AF = mybir.ActivationFunctionType
    OP = mybir.AluOpType

    # The act-table placement pass picks the first set containing each
    # function; Exp/Ln then ping-pong between exp_and_others and natural_log
    # (one ~2.7us table load per activation). Make Exp/Ln resolvable only in
    # the combined set so a single load serves the whole kernel. Positions in
    # the dict are preserved (set ids index act_info.json order).
    import concourse.bacc as bacc_mod

    orig_get_tables = bacc_mod.get_activation_tables

    def _edited_tables(arch):
        edited = {}
        for name, fns in orig_get_tables(arch).items():
            fns = set(fns)
            if name != "natural_log_exp_and_others":
                fns.discard(AF.Exp)
                fns.discard(AF.Ln)
            edited[name] = fns
        return edited

    bacc_mod.get_activation_tables = _edited_tables

    bf16 = mybir.dt.bfloat16
    split = mm_dtype_name == "bf16split"

    nc = bacc.Bacc(
        "TRN2", target_bir_lowering=False, debug=False, num_devices=N_CORES
    )
    if split:
        xt_hi = nc.dram_tensor("xt_hi", [D, T], bf16, kind="ExternalInput")
        xt_lo = nc.dram_tensor("xt_lo", [D, T], bf16, kind="ExternalInput")
        wc_hi = nc.dram_tensor("wc_hi", [128, 8, 128], bf16, kind="ExternalInput")
        wc_lo = nc.dram_tensor("wc_lo", [128, 8, 128], bf16, kind="ExternalInput")
    else:
        xt = nc.dram_tensor("xt", [D, T], f32, kind="ExternalInput")
        wc = nc.dram_tensor("wc", [128, 8, 128], f32, kind="ExternalInput")
    nzT = nc.dram_tensor("nzT", [E, T], f32, kind="ExternalInput")
    iota = nc.dram_tensor("iota", [128, E], f32, kind="ExternalInput")
    ident = nc.dram_tensor("ident", [128, 128], f32, kind="ExternalInput")
    bn = nc.dram_tensor("bn", [E, 1], f32, kind="ExternalInput")
    br = nc.dram_tensor("br", [E, 1], f32, kind="ExternalInput")
    out_r = nc.dram_tensor("out_r", [128, G, E], f32, kind="ExternalOutput")
    out_i = nc.dram_tensor("out_i", [128, G, 2], u32, kind="ExternalOutput")

    with tile.TileContext(nc) as tc, ExitStack() as ctx:
        const = ctx.enter_context(tc.tile_pool(name="const", bufs=1))
        xp = ctx.enter_context(tc.tile_pool(name="xp", bufs=xp_bufs))
        nzp = ctx.enter_context(tc.tile_pool(name="nzp", bufs=2))
        work = ctx.enter_context(tc.tile_pool(name="work", bufs=work_bufs))
        outp = ctx.enter_context(tc.tile_pool(name="outp", bufs=2))
        smalls = ctx.enter_context(tc.tile_pool(name="smalls", bufs=2))
        stat = ctx.enter_context(tc.tile_pool(name="stat", bufs=1))
        psA = ctx.enter_context(tc.tile_pool(name="psA", bufs=ps_bufs, space="PSUM"))
        psB = ctx.enter_context(tc.tile_pool(name="psB", bufs=ps_bufs, space="PSUM"))

        if split:
            wch_t = const.tile([128, 8, 128], bf16)
            nc.sync.dma_start(wch_t[:], wc_hi.ap())
            wcl_t = const.tile([128, 8, 128], bf16)
            nc.sync.dma_start(wcl_t[:], wc_lo.ap())
        else:
            wc_t = const.tile([128, 8, 128], f32)
            nc.sync.dma_start(wc_t[:], wc.ap())
        iota_t = const.tile([128, E], f32)
        nc.sync.dma_start(iota_t[:], iota.ap())
        ident_t = const.tile([128, 128], f32)
        nc.sync.dma_start(ident_t[:], ident.ap())
        bn_t = const.tile([E, 1], f32)
        nc.sync.dma_start(bn_t[:], bn.ap())
        br_t = const.tile([E, 1], f32)
        nc.sync.dma_start(br_t[:], br.ap())

        maxv = stat.tile([128, G, 8], f32)
        maxi = stat.tile([128, G, 8], u32)

        # [1024, T] viewed as [128 partitions, 8 d-chunks, T]
        if split:
            xth_ap = xt_hi.ap().rearrange("(c p) t -> p c t", p=128)
            xtl_ap = xt_lo.ap().rearrange("(c p) t -> p c t", p=128)
        else:
            xt_ap = xt.ap().rearrange("(c p) t -> p c t", p=128)

        noisy_tiles = {}

        def phase_a(ti):
            t0 = ti * TT
            if split:
                xtile_h = xp.tile([128, 8, TT], bf16, tag="xh")
                xtile_l = xp.tile([128, 8, TT], bf16, tag="xl")
                step = max(1, 8 // split_x)
                for sx in range(0, 8, step):
                    nc.sync.dma_start(
                        xtile_h[:, sx : sx + step, :],
                        xth_ap[:, sx : sx + step, t0 : t0 + TT],
                    )
                    nc.sync.dma_start(
                        xtile_l[:, sx : sx + step, :],
                        xtl_ap[:, sx : sx + step, t0 : t0 + TT],
                    )
            else:
                xtile = xp.tile([128, 8, TT], f32)
                if split_x == 1:
                    nc.sync.dma_start(xtile[:], xt_ap[:, :, t0 : t0 + TT])
                else:
                    step = 8 // split_x
                    for sx in range(split_x):
                        nc.sync.dma_start(
                            xtile[:, sx * step : (sx + 1) * step, :],
                            xt_ap[:, sx * step : (sx + 1) * step, t0 : t0 + TT],
                        )
            ntile = nzp.tile([E, TT], f32)
            nc.sync.dma_start(ntile[:], nzT.ap()[:, t0 : t0 + TT])

            if ablate == "dma_only":
                return
            # logits: psum rows 0:64 = route, 64:128 = noise
            ps_a = psA.tile([128, TT], f32)
            if split:
                nmm = 0
                for c in range(8):
                    for w_t, x_t in (
                        (wch_t, xtile_h),
                        (wcl_t, xtile_h),
                        (wch_t, xtile_l),
                    ):
                        nc.tensor.matmul(
                            ps_a[:],
                            lhsT=w_t[:, c, :],
                            rhs=x_t[:, c, :],
                            start=(nmm == 0),
                            stop=(nmm == 23),
                        )
                        nmm += 1
            else:
                for c in range(8):
                    nc.tensor.matmul(
                        ps_a[:],
                        lhsT=wc_t[:, c, :].bitcast(mm_dt),
                        rhs=xtile[:, c, :].bitcast(mm_dt),
                        start=(c == 0),
                        stop=(c == 7),
                    )

            if ablate == "mm_only":
                return
            # noise logits to an SBUF tile at partition base 0
            # (single-src 64-channel cross-quadrant move: HW-verified path)
            nz_sb = work.tile([E, TT], f32, tag="nz")
            nc.vector.tensor_copy(nz_sb[:], ps_a[E:128, :])
            # softplus(nz + b_noise) = Ln(1 + Exp(nz + b_noise))
            e_t = work.tile([E, TT], f32, tag="e")
            nc.scalar.activation(e_t[:], nz_sb[:], AF.Exp, bias=bn_t[:])
            sp_t = work.tile([E, TT], f32, tag="sp")
            nc.scalar.activation(sp_t[:], e_t[:], AF.Ln, bias=1.0)
            nm_t = work.tile([E, TT], f32, tag="nm")
            nc.vector.tensor_mul(nm_t[:], ntile[:], sp_t[:])
            # noisy rows 0:64 = (route + b_route) + noise*softplus
            noisy = work.tile([128, TT], f32, tag="noisy")
            nc.gpsimd.memset(noisy[E:128, :], 0.0)
            nc.vector.scalar_tensor_tensor(
                noisy[0:E, :],
                in0=ps_a[0:E, :],
                scalar=br_t[:],
                in1=nm_t[:],
                op0=OP.add,
                op1=OP.add,
            )

            if ablate == "noisy":
                noisy_tiles[ti] = None
                return
            noisy_tiles[ti] = noisy

        def phase_b(ti):
            noisy = noisy_tiles.pop(ti)
            # transpose to token-major via PE: 4 blocks of [128,128]
            ps_b = psB.tile([128, TT], f32)
            for t4 in range(4):
                nc.tensor.transpose(
                    ps_b[:, t4 * 128 : (t4 + 1) * 128],
                    noisy[:, t4 * 128 : (t4 + 1) * 128],
                    ident_t[:],
                )
            # gather the valid expert columns: [128 tok, 4 grp, 64 experts]
            ntok = work.tile([128, 4, E], f32, tag="ntok")
            ps_b4 = ps_b[:].rearrange("p (b m) -> p b m", m=128)
            if ntok_eng == "scalar":
                nc.scalar.copy(ntok[:], ps_b4[:, :, 0:E])
            else:
                getattr(nc, ntok_eng).tensor_copy(ntok[:], ps_b4[:, :, 0:E])

            if ablate == "transp":
                return
            for g in range(4):
                Gi = ti * 4 + g
                nc.vector.max(out=maxv[:, Gi], in_=ntok[:, g])
                nc.vector.max_index(
                    out=maxi[:, Gi], in_max=maxv[:, Gi], in_values=ntok[:, g]
                )

            gs = slice(ti * 4, ti * 4 + 4)
            # 2-way softmax over (v1, v2): p1 = 1/(1+exp(v2-v1)), p2 = 1-p1
            d_t = smalls.tile([128, 4, 1], f32, tag="d")
            nc.vector.tensor_sub(d_t[:], maxv[:, gs, 1:2], maxv[:, gs, 0:1])
            e2_t = smalls.tile([128, 4, 1], f32, tag="e2")
            nc.scalar.activation(e2_t[:], d_t[:], AF.Exp)
            s2_t = smalls.tile([128, 4, 1], f32, tag="s2")
            nc.vector.tensor_scalar_add(s2_t[:], e2_t[:], 1.0)
            p1_t = smalls.tile([128, 4, 1], f32, tag="p1")
            nc.vector.reciprocal(p1_t[:], s2_t[:])
            p2_t = smalls.tile([128, 4, 1], f32, tag="p2")
            nc.vector.tensor_mul(p2_t[:], e2_t[:], p1_t[:])
            idxf = smalls.tile([128, 4, 2], f32, tag="idxf")
            nc.vector.tensor_copy(idxf[:], maxi[:, gs, 0:2])

            if ablate == "topk":
                return
            # scatter: out[t, e] = p1*(e==i1) + p2*(e==i2)
            rt = outp.tile([128, 4, E], f32, tag="rt")
            sc = outp.tile([128, E], f32, tag="sc")
            seng = getattr(nc, scat_eng)
            for g in range(4):
                seng.tensor_scalar(
                    rt[:, g],
                    iota_t[:],
                    idxf[:, g, 0:1],
                    p1_t[:, g],
                    op0=OP.is_equal,
                    op1=OP.mult,
                )
                seng.tensor_scalar(
                    sc[:],
                    iota_t[:],
                    idxf[:, g, 1:2],
                    p2_t[:, g],
                    op0=OP.is_equal,
                    op1=OP.mult,
                )
                seng.tensor_add(rt[:, g], rt[:, g], sc[:])
            nc.scalar.dma_start(out_r.ap()[:, gs, :], rt[:])

        def emit_all():
            for ti in range(NT):
                phase_a(ti)
                if ablate in ("dma_only", "mm_only", "noisy"):
                    continue
                if ti >= lag:
                    phase_b(ti - lag)
            if ablate not in ("dma_only", "mm_only", "noisy"):
                for ti in range(NT - lag, NT):
                    phase_b(ti)

            if ablate in ("full", "topk"):
                idxp = stat.tile([128, G, 2], u32)
                nc.vector.tensor_copy(idxp[:], maxi[:, :, 0:2])
                nc.scalar.dma_start(out_i.ap(), idxp[:])

        if reps:
            with tc.For_i(0, reps, 1):
                emit_all()
        else:
            emit_all()

    try:
        if do_compile:
            nc.compile()
    finally:
        bacc_mod.get_activation_tables = orig_get_tables
    return nc


def _get_nc():
    mm_dtype = os.environ.get("NOISYTOPK_MM_DTYPE", "float32")
    key = ("nc", mm_dtype)
    if key not in _cache:
        _cache[key] = _build(mm_dtype, lag=1, split_x=8)
    return _cache[key]


def _split_bf16(a):
    import ml_dtypes

    hi = a.astype(ml_dtypes.bfloat16)
    lo = (a - hi.astype(np.float32)).astype(ml_dtypes.bfloat16)
    return hi, lo


def kernel(mh_output, noise, W_route, b_route, W_noise, b_noise):
    from concourse.bass_utils import run_bass_kernel_spmd

    mm_dtype = os.environ.get("NOISYTOPK_MM_DTYPE", "float32")
    split = mm_dtype == "bf16split"
    nc = _get_nc()

    mh_output = np.ascontiguousarray(mh_output, dtype=np.float32)
    noise = np.ascontiguousarray(noise, dtype=np.float32)
    wcat_T = np.concatenate(
        [np.asarray(W_route, np.float32), np.asarray(W_noise, np.float32)], axis=0
    ).T  # [1024, 128]
    wc_host = np.ascontiguousarray(
        wcat_T.reshape(8, 128, 128).transpose(1, 0, 2)
    )  # [128 part, 8 chunk, 128]
    iota_host = np.ascontiguousarray(
        np.broadcast_to(np.arange(E, dtype=np.float32), (128, E))
    )
    ident_host = np.eye(128, dtype=np.float32)
    bn_host = np.ascontiguousarray(np.asarray(b_noise, np.float32).reshape(E, 1))
    br_host = np.ascontiguousarray(np.asarray(b_route, np.float32).reshape(E, 1))
    if split:
        wch_host, wcl_host = _split_bf16(wc_host)

    in_maps = []
    for i in range(N_CORES):
        m = {
            "nzT": np.ascontiguousarray(noise[i].T),  # [64, 4096]
            "iota": iota_host,
            "ident": ident_host,
            "bn": bn_host,
            "br": br_host,
        }
        xt_i = np.ascontiguousarray(mh_output[i].T)  # [1024, 4096]
        if split:
            xh, xl = _split_bf16(xt_i)
            m["xt_hi"], m["xt_lo"] = xh, xl
            m["wc_hi"], m["wc_lo"] = wch_host, wcl_host
        else:
            m["xt"] = xt_i
            m["wc"] = wc_host
        in_maps.append(m)

    res = run_bass_kernel_spmd(nc, in_maps, core_ids=list(range(N_CORES)))

    router = np.empty((N_CORES, T, E), dtype=np.float32)
    indices = np.empty((N_CORES, T, 2), dtype=np.int32)
    for i in range(N_CORES):
        r = res.results[i]["out_r"]  # [128, G, E]
        ix = res.results[i]["out_i"]  # [128, G, 2] uint32
        router[i] = r.transpose(1, 0, 2).reshape(T, E)
        indices[i] = ix.transpose(1, 0, 2).reshape(T, 2).view(np.int32)
    return router, indices
